# revision 25
# baseline (speedup 1.0000x reference)
"""Trainium2 Bass kernel for nn_AttentionBase (channel attention with conv qkv).

Math restructuring (validated in fp64/fp32 numpy vs the jax reference):
  - conv1 (1x1) folds into conv2 (k=3): C_k = W2[:,:,k] @ W1  -> one k=3 conv.
  - q,k are never materialized. The per-head 16x16 channel-attention matrix
    only needs G_qk / diag(G_qq) / diag(G_kk), and all of those are sandwiches
    of the lag-autocorrelations of x:
        M_d = sum_t x_t x_{t+d}^T   (d = 0,1,2)
        G_qk = sum_{k,l} Cq_k M_{l-k} Ck_l^T  - two rank-1 boundary terms
    so pass 1 is just per-tile PE transposes of x (via identity matmul) plus
    one accumulating Gram matmul per 128-token tile: 6 PE cycles/token vs 9
    for the explicit conv+gram+sumsq formulation.
  - v is never materialized:  out = Wp @ BlockDiag(A) @ v = conv(x, M @ V_k)
    with M = Wp @ BlockDiag(A) computed on-device (tiny matmuls).

Per core (1 batch element per core, 8 cores):
  pass 1: 128 tiles: 3 shifted PE transposes (bf16, exact) -> [xT0|xT1|xT2],
          one Gram matmul accumulating [M0|M1|M2] in PSUM.
  epilogue: sandwich matmuls recover G_qk/G_qq/G_kk; norms via
          exp(-0.5*ln(ss)); rank-1 rescale; per-head softmax; folded weights.
  pass 2: k=3 conv of x (bf16) with folded weights -> output.
"""

import sys

import numpy as np

sys.path.insert(0, "/opt/trn_rl_repo")

import ml_dtypes  # noqa: E402

import concourse.bass as bass  # noqa: E402
import concourse.tile as tile  # noqa: E402
from concourse import bacc, mybir  # noqa: E402
from concourse.bass_utils import run_bass_kernel_spmd  # noqa: E402

F32 = mybir.dt.float32
BF16 = mybir.dt.bfloat16

B, C, N = 8, 128, 16384
HEADS, HD = 8, 16
NCORES = 8
CHUNK = 1024          # DMA / bf16-convert chunk (tokens)
NT = N // 128         # pass-1 token tiles
T2 = 512              # pass-2 token tile
PADW = N + 4          # col j of xbf <-> x[:, j-1]; col 0 and cols N+1.. are 0
AFT = mybir.ActivationFunctionType


def build_program():
    nc = bacc.Bacc(None, target_bir_lowering=False)

    x_d = nc.dram_tensor("x", [C, N], F32, kind="ExternalInput")
    ident_d = nc.dram_tensor("ident", [C, C], BF16, kind="ExternalInput")
    idf_d = nc.dram_tensor("idf", [C, C], F32, kind="ExternalInput")
    cqt_d = nc.dram_tensor("cqt", [C, 3 * C], BF16, kind="ExternalInput")
    ckt_d = nc.dram_tensor("ckt", [C, 3 * C], BF16, kind="ExternalInput")
    s1rhs_d = nc.dram_tensor("s1rhs", [C, 3 * 256], BF16, kind="ExternalInput")
    wv_d = nc.dram_tensor("wv", [C, 3 * C], BF16, kind="ExternalInput")
    wpt_d = nc.dram_tensor("wpt", [C, C], BF16, kind="ExternalInput")
    svec_d = nc.dram_tensor("svec", [1, C], F32, kind="ExternalInput")
    mask_d = nc.dram_tensor("mask", [C, C], F32, kind="ExternalInput")
    out_d = nc.dram_tensor("out", [C, N], F32, kind="ExternalOutput")

    with tile.TileContext(nc) as tc:
        with (
            tc.tile_pool(name="const", bufs=1) as const,
            tc.tile_pool(name="xpool", bufs=1) as xpool,
            tc.tile_pool(name="work", bufs=3) as work,
            tc.tile_pool(name="epi", bufs=1) as epi,
            tc.tile_pool(name="psum", bufs=1, space="PSUM") as psum,
        ):
            # ---- constants needed by pass 1 ----
            ident_sb = const.tile([C, C], BF16)
            nc.scalar.dma_start(out=ident_sb, in_=ident_d[:, :])
            ones_sb = const.tile([C, 1], BF16)
            nc.vector.memset(ones_sb, 1.0)
            zeros_sb = const.tile([C, C], BF16)
            nc.vector.memset(zeros_sb, 0.0)
            # Pre-warm the PE's HAM clock gate during the DMA prologue:
            # dependency-free full-width zero matmuls keep the activity
            # window busy so pass 1 starts at 2.4 GHz instead of 1.2.
            warm_ps = psum.tile([C, C], F32, tag="mps", padded_shape=[C, 3 * C])
            for _ in range(56):
                nc.tensor.matmul(warm_ps, lhsT=zeros_sb, rhs=zeros_sb)

            def warm_burst(tie_ap, n=4):
                # one matmul gated on `tie_ap` (so it fires mid-epilogue, not
                # early), then dependency-free zero matmuls that run back to
                # back -- bridges PE-idle stretches that would re-throttle HAM
                wb_ps = psum.tile([C, C], F32, tag="mps", name="wb_ps")
                kp = tie_ap.shape[0]
                nc.tensor.matmul(
                    wb_ps[0 : tie_ap.shape[1], :],
                    lhsT=tie_ap,
                    rhs=zeros_sb[0:kp, :],
                )
                for _ in range(n - 1):
                    nc.tensor.matmul(wb_ps, lhsT=zeros_sb, rhs=zeros_sb)

            # Pin ONE activation table set covering every ACT function used
            # (square/ln/exp/copy all live in natural_log_exp_and_others) so
            # no mid-kernel table reloads land on the epilogue critical path.
            from concourse.hw_specs import get_activation_tables

            tables = get_activation_tables(nc.m.arch)
            set_id = list(tables).index("natural_log_exp_and_others")
            need = {AFT.Square, AFT.Ln, AFT.Exp, AFT.Copy}
            assert need <= tables["natural_log_exp_and_others"], (
                tables["natural_log_exp_and_others"]
            )
            nc.scalar.add_instruction(
                mybir.InstLoadActFuncSet(
                    name=nc.get_next_instruction_name(),
                    ins=[],
                    outs=[],
                    act_func_set_id=set_id,
                )
            )

            # ---- x resident in SBUF as bf16, padded [C, PADW] ----
            xbf_sb = xpool.tile([C, PADW], BF16)
            nc.vector.memset(xbf_sb[:, 0:1], 0.0)
            nc.vector.memset(xbf_sb[:, N + 1 : PADW], 0.0)
            bounds = [0, 160, 320, 640, 1152]
            while bounds[-1] < N:
                bounds.append(min(N, bounds[-1] + CHUNK))
            for ci in range(len(bounds) - 1):
                a, bnd = bounds[ci], bounds[ci + 1]
                stg = work.tile([C, CHUNK], F32, tag="stg")
                nc.sync.dma_start(out=stg[:, 0 : bnd - a], in_=x_d[:, a:bnd])
                # split big casts into 512-token pieces: a cast insertion in
                # the in-order DVE/ACT queue then blocks the xt-copy cadence
                # (which gates the PE via PSUM-buffer reuse) for at most
                # ~0.55us instead of ~0.85us
                pieces = (
                    [(0, bnd - a)]
                    if bnd - a <= 512
                    else [(0, 512), (512, bnd - a)]
                )
                for lo, hi in pieces:
                    if ci % 2 == 0:
                        nc.vector.tensor_copy(
                            out=xbf_sb[:, 1 + a + lo : 1 + a + hi],
                            in_=stg[:, lo:hi],
                        )
                    else:
                        nc.scalar.copy(
                            out=xbf_sb[:, 1 + a + lo : 1 + a + hi],
                            in_=stg[:, lo:hi],
                        )

            # ---- pass 1: per-tile shifted transposes + lag-Gram accumulate --
            # tile m covers tokens [128m, 128m+128); transpose d gives rows
            # p -> x_{128m+p+d}. Gram matmul lhsT=xT0, rhs=[xT0|xT1|xT2]
            # accumulates [M0|M1|M2] with M_d = sum_t x_t x_{t+d}^T.
            LAG = 5
            m_ps = psum.tile([C, 3 * C], F32, tag="mps")
            hist = {}
            epi_loaded = False
            for m in range(NT + LAG):
                if m == 8 and not epi_loaded:
                    # epilogue-only weights: issued mid-pass-1 on the gpsimd
                    # queue so they stay off the prologue's critical path
                    epi_loaded = True
                    idf_sb = const.tile([C, C], F32)
                    nc.gpsimd.dma_start(out=idf_sb, in_=idf_d[:, :])
                    cqt_sb = const.tile([C, 3 * C], BF16)
                    nc.gpsimd.dma_start(out=cqt_sb, in_=cqt_d[:, :])
                    ckt_sb = const.tile([C, 3 * C], BF16)
                    nc.gpsimd.dma_start(out=ckt_sb, in_=ckt_d[:, :])
                    s1rhs_sb = const.tile([C, 3 * 256], BF16)
                    nc.gpsimd.dma_start(out=s1rhs_sb, in_=s1rhs_d[:, :])
                    wv_sb = const.tile([C, 3 * C], BF16)
                    nc.gpsimd.dma_start(out=wv_sb, in_=wv_d[:, :])
                    wpt_sb = const.tile([C, C], BF16)
                    nc.gpsimd.dma_start(out=wpt_sb, in_=wpt_d[:, :])
                    svec_sb = const.tile([1, C], F32)
                    nc.gpsimd.dma_start(out=svec_sb, in_=svec_d[:, :])
                    mask_sb = const.tile([C, C], F32)
                    nc.gpsimd.dma_start(out=mask_sb, in_=mask_d[:, :])
                if m < NT:
                    o = 1 + 128 * m
                    tp_ps = psum.tile([C, T2], F32, tag="tp", bufs=4)
                    for dd in range(3):
                        nc.tensor.matmul(
                            tp_ps[:, dd * C : (dd + 1) * C],
                            lhsT=xbf_sb[:, o + dd : o + dd + 128],
                            rhs=ident_sb,
                            start=True,
                            stop=True,
                        )
                    xt_sb = work.tile([C, 3 * C], BF16, tag="xt", bufs=8)
                    if m % 2 == 0:
                        nc.vector.tensor_copy(out=xt_sb, in_=tp_ps[:, 0 : 3 * C])
                    else:
                        nc.scalar.copy(out=xt_sb, in_=tp_ps[:, 0 : 3 * C])
                    hist[m] = xt_sb
                if m == NT:
                    # boundary row vectors [yq0 | yk0 | yq2 | yk2]:
                    # yq0 = (Cq_0 x_{N-1})^T etc. Needs the last x chunk, so
                    # issued after the final tile's transposes.
                    brow_ps = psum.tile([1, 4 * C], F32, tag="epiA")
                    for i, (col, blk, wsb) in enumerate(
                        [
                            (N, 0, cqt_sb),
                            (N, 0, ckt_sb),
                            (1, 2, cqt_sb),
                            (1, 2, ckt_sb),
                        ]
                    ):
                        nc.tensor.matmul(
                            brow_ps[:, i * C : (i + 1) * C],
                            lhsT=xbf_sb[:, col : col + 1],
                            rhs=wsb[:, blk * C : (blk + 1) * C],
                            start=True,
                            stop=True,
                        )
                    brow_sb = epi.tile([1, 4 * C], BF16)
                    nc.vector.tensor_copy(out=brow_sb, in_=brow_ps)
                    bneg_sb = epi.tile([1, 4 * C], BF16)
                    nc.vector.tensor_scalar_mul(bneg_sb, brow_sb, -1.0)
                if m >= LAG:
                    q = hist.pop(m - LAG)
                    nc.tensor.matmul(
                        m_ps,
                        lhsT=q[:, 0:C],
                        rhs=q,
                        start=(m - LAG == 0),
                        stop=(m - LAG == NT - 1),
                    )

            # ---- epilogue stage 0: M -> bf16, transpose M1, M2 ----
            mb_sb = epi.tile([C, 3 * C], BF16)
            nc.vector.tensor_copy(out=mb_sb[:, 0:192], in_=m_ps[:, 0:192])
            nc.scalar.copy(out=mb_sb[:, 192:384], in_=m_ps[:, 192:384])
            mt_ps = psum.tile([C, 2 * C], F32, tag="epiB")
            nc.tensor.matmul(
                mt_ps[:, 0:C], lhsT=mb_sb[:, C : 2 * C], rhs=ident_sb,
                start=True, stop=True,
            )
            nc.tensor.matmul(
                mt_ps[:, C : 2 * C], lhsT=mb_sb[:, 2 * C : 3 * C], rhs=ident_sb,
                start=True, stop=True,
            )
            mtb_sb = epi.tile([C, 2 * C], BF16)
            nc.scalar.copy(out=mtb_sb, in_=mt_ps)
            warm_burst(mb_sb[:, 0:C], 4)

            # stage 1: [R_k | P_k] = sum_l M_{l-k} [CkT_l | CqT_l]
            # lhsT must be M_{l-k}^T: delta>=0 -> mtb blocks; delta<0 -> mb.
            def s1_lhs(delta):
                if delta == 0:
                    return mb_sb[:, 0:C]
                if delta > 0:
                    return mtb_sb[:, (delta - 1) * C : delta * C]
                return mb_sb[:, -delta * C : (-delta + 1) * C]

            # k=2 needs no mtb (deltas -2,-1,0): issue first to overlap the
            # mtb cast; then k=1 (one mtb term), then k=0.
            rp_tag = {0: "epiA", 1: "epiB", 2: "epiC"}
            rp_ps = {}
            rp_sb = {}
            for k in (2, 1, 0):
                rp_ps[k] = psum.tile(
                    [C, 256], F32, tag=rp_tag[k], name=f"rp{k}_ps"
                )
                for i, l in enumerate((0, 1, 2) if k != 1 else (0, 2, 1)):
                    nc.tensor.matmul(
                        rp_ps[k],
                        lhsT=s1_lhs(l - k),
                        rhs=s1rhs_sb[:, l * 256 : (l + 1) * 256],
                        start=(i == 0),
                        stop=(i == 2),
                    )
                rp_sb[k] = epi.tile([C, 256], BF16, name=f"rp{k}_sb")
                if k == 2:
                    nc.vector.tensor_copy(out=rp_sb[k], in_=rp_ps[k])
                else:
                    nc.scalar.copy(out=rp_sb[k], in_=rp_ps[k])

            # stage 2: [G_qk | G_qq] and G_kk, minus rank-1 boundary terms
            gq_ps = psum.tile([C, 256], F32, tag="epiA")
            for k in range(3):
                nc.tensor.matmul(
                    gq_ps,
                    lhsT=cqt_sb[:, k * C : (k + 1) * C],
                    rhs=rp_sb[k],
                    start=(k == 0),
                    stop=False,
                )
            # G_qk -= yq0 yk0^T + yq2 yk2^T ; G_qq -= yq0 yq0^T + yq2 yq2^T
            nc.tensor.matmul(
                gq_ps[:, 0:C], lhsT=bneg_sb[:, 0:C], rhs=brow_sb[:, C : 2 * C],
                start=False, stop=False,
            )
            nc.tensor.matmul(
                gq_ps[:, 0:C], lhsT=bneg_sb[:, 2 * C : 3 * C],
                rhs=brow_sb[:, 3 * C : 4 * C], start=False, stop=True,
            )
            nc.tensor.matmul(
                gq_ps[:, C : 2 * C], lhsT=bneg_sb[:, 0:C], rhs=brow_sb[:, 0:C],
                start=False, stop=False,
            )
            nc.tensor.matmul(
                gq_ps[:, C : 2 * C], lhsT=bneg_sb[:, 2 * C : 3 * C],
                rhs=brow_sb[:, 2 * C : 3 * C], start=False, stop=True,
            )
            gk_ps = psum.tile([C, C], F32, tag="epiC")
            for k in range(3):
                nc.tensor.matmul(
                    gk_ps,
                    lhsT=ckt_sb[:, k * C : (k + 1) * C],
                    rhs=rp_sb[k][:, 0:C],
                    start=(k == 0),
                    stop=False,
                )
            nc.tensor.matmul(
                gk_ps, lhsT=bneg_sb[:, C : 2 * C], rhs=brow_sb[:, C : 2 * C],
                start=False, stop=False,
            )
            nc.tensor.matmul(
                gk_ps, lhsT=bneg_sb[:, 3 * C : 4 * C],
                rhs=brow_sb[:, 3 * C : 4 * C], start=False, stop=True,
            )

            # norms: ss_q = diag(G_qq), ss_k = diag(G_kk) via identity mask +
            # ones-colsum matmul -> [1, 2C] row
            dqk_sb = epi.tile([C, 2 * C], BF16)
            nc.vector.tensor_mul(dqk_sb[:, 0:C], gq_ps[:, C : 2 * C], idf_sb)
            nc.vector.tensor_mul(dqk_sb[:, C : 2 * C], gk_ps, idf_sb)
            ss_ps = psum.tile([1, 2 * C], F32, tag="epiB", name="ss_ps")
            nc.tensor.matmul(ss_ps, lhsT=ones_sb, rhs=dqk_sb, start=True, stop=True)
            warm_burst(dqk_sb[:, 0:C], 6)
            # r = rsqrt(ss) via exp(-0.5*ln(ss)); ss is a large positive
            # sum of squares so no clamp is needed, and ACT reads it straight
            # from PSUM (saves a copy + a clamp on the serial chain)
            ss_sb = epi.tile([1, 2 * C], F32)
            nc.scalar.activation(ss_sb, ss_ps, AFT.Ln)
            # tiny bf16 matmuls tied to epilogue intermediates keep the PE's
            # HAM activity window warm through the serial epilogue ops
            ssb_sb = epi.tile([1, 2 * C], BF16)
            nc.vector.tensor_copy(out=ssb_sb, in_=ss_sb)
            warm_burst(ssb_sb[:, 0:C], 6)
            r_sb = epi.tile([1, 2 * C], F32)
            nc.scalar.activation(r_sb, ss_sb, AFT.Exp, scale=-0.5)
            rq_sb = epi.tile([1, C], F32)
            nc.vector.tensor_mul(rq_sb, r_sb[:, 0:C], svec_sb)
            rqb_sb = epi.tile([1, C], BF16)
            nc.vector.tensor_copy(out=rqb_sb, in_=rq_sb)
            rkb_sb = epi.tile([1, C], BF16)
            nc.vector.tensor_copy(out=rkb_sb, in_=r_sb[:, C : 2 * C])
            warm_burst(rqb_sb, 6)

            # bf16 rank-1 (fp32 matmuls lower to LOW/HIGH double passes)
            outer_ps = psum.tile([C, C], F32, tag="epiB")
            nc.tensor.matmul(outer_ps, lhsT=rqb_sb, rhs=rkb_sb)
            outer_sb = epi.tile([C, C], F32)
            nc.vector.tensor_copy(out=outer_sb, in_=outer_ps)

            # A = softmax over each 16x16 diagonal block; the additive mask is
            # -1e30 off-block, so exp underflows to exactly 0 there. The row
            # max is taken over the unmasked row (softmax is shift-invariant
            # and |A|<=1, so any in-range shift is numerically fine).
            a_sb = epi.tile([C, C], F32)
            nc.vector.tensor_mul(a_sb, gq_ps[:, 0:C], outer_sb)
            nc.vector.tensor_add(a_sb, a_sb, mask_sb)
            rsum = epi.tile([C, 1], F32)
            ae_sb = epi.tile([C, C], BF16)
            nc.scalar.activation(ae_sb, a_sb, AFT.Exp)
            warm_burst(ae_sb[:, 0:C], 6)
            nc.vector.reduce_sum(out=rsum, in_=ae_sb, axis=mybir.AxisListType.X)
            nc.vector.reciprocal(rsum, rsum)
            wptn_sb = epi.tile([C, C], BF16)
            nc.vector.tensor_scalar_mul(wptn_sb, wpt_sb, rsum)

            # MT[d, m] = sum_c A[c, d] * WpT[c, m]
            mtm_ps = psum.tile([C, C], F32, tag="epiC")
            nc.tensor.matmul(mtm_ps, lhsT=ae_sb, rhs=wptn_sb)
            mtm_sb = epi.tile([C, C], BF16)
            nc.vector.tensor_copy(out=mtm_sb, in_=mtm_ps)
            warm_burst(mtm_sb, 4)

            foldT_sb = epi.tile([C, 3 * C], BF16)
            fold_tag = {0: "epiC", 1: "epiB", 2: "epiC"}
            for k in range(3):
                fold_ps = psum.tile([C, C], F32, tag=fold_tag[k], name=f"fold{k}")
                nc.tensor.matmul(
                    fold_ps, lhsT=wv_sb[:, k * C : (k + 1) * C], rhs=mtm_sb
                )
                nc.vector.tensor_copy(
                    out=foldT_sb[:, k * C : (k + 1) * C], in_=fold_ps
                )

            # ---- pass 2: folded k=3 conv of x (bf16), channel-major ----
            for j in range(N // T2):
                o_ps = psum.tile([C, T2], F32, tag="tp", bufs=4)
                for k in range(3):
                    o = j * T2 + k
                    nc.tensor.matmul(
                        o_ps,
                        lhsT=foldT_sb[:, k * C : (k + 1) * C],
                        rhs=xbf_sb[:, o : o + T2],
                        start=(k == 0),
                        stop=(k == 2),
                    )
                o_sb = work.tile([C, T2], F32, tag="o_sb", bufs=8)
                if j % 2 == 0:
                    nc.vector.tensor_copy(out=o_sb, in_=o_ps)
                else:
                    nc.scalar.copy(out=o_sb, in_=o_ps)
                nc.sync.dma_start(
                    out=out_d[:, j * T2 : (j + 1) * T2], in_=o_sb
                )

    nc.finalize()
    return nc


def _prep_weights(w_qkv1, w_qkv2, w_proj, scale):
    W1 = np.asarray(w_qkv1, np.float32)[:, :, 0]          # [384, 128]
    W2 = np.asarray(w_qkv2, np.float32)                   # [384, 384, 3]
    Ck = np.stack([W2[:, :, k] @ W1 for k in range(3)])   # [3, 384, 128]
    Qk, Kk, Vk = Ck[:, 0:C, :], Ck[:, C : 2 * C, :], Ck[:, 2 * C :, :]
    bf = ml_dtypes.bfloat16
    cqt = np.concatenate([Qk[k].T for k in range(3)], axis=1)   # [128, 384]
    ckt = np.concatenate([Kk[k].T for k in range(3)], axis=1)
    s1rhs = np.concatenate(
        [np.concatenate([Kk[l].T, Qk[l].T], axis=1) for l in range(3)], axis=1
    )                                                      # [128, 768]
    wv = np.concatenate([Vk[k] for k in range(3)], axis=1)  # [128, 384]
    wpt = np.ascontiguousarray(np.asarray(w_proj, np.float32)[:, :, 0].T)
    svec = np.repeat(np.asarray(scale, np.float32)[:, 0, 0], HD)[None, :]
    mask = np.full((C, C), -1e30, np.float32)
    for h in range(HEADS):
        mask[h * HD : (h + 1) * HD, h * HD : (h + 1) * HD] = 0.0
    return {
        "ident": np.eye(C, dtype=bf),
        "idf": np.eye(C, dtype=np.float32),
        "cqt": np.ascontiguousarray(cqt).astype(bf),
        "ckt": np.ascontiguousarray(ckt).astype(bf),
        "s1rhs": np.ascontiguousarray(s1rhs).astype(bf),
        "wv": np.ascontiguousarray(wv).astype(bf),
        "wpt": wpt.astype(bf),
        "svec": np.ascontiguousarray(svec, np.float32),
        "mask": mask,
    }


_CACHE = {}


def kernel(x, w_qkv1, w_qkv2, w_proj, scale, _trace=False, _tmpdir=None):
    x = np.asarray(x, np.float32)
    assert x.shape == (B, C, N), x.shape
    wmap = _prep_weights(w_qkv1, w_qkv2, w_proj, scale)

    if "nc" not in _CACHE:
        _CACHE["nc"] = build_program()
    nc = _CACHE["nc"]

    in_maps = [
        {"x": np.ascontiguousarray(x[i]), **wmap}
        for i in range(NCORES)
    ]
    res = run_bass_kernel_spmd(
        nc,
        in_maps,
        core_ids=list(range(NCORES)),
        trace=_trace,
        tmpdir=_tmpdir,
    )
    out = np.stack([r["out"] for r in res.results]).astype(np.float32)
    if _trace:
        _CACHE["last_result"] = res
    return out


# revision 27
# speedup vs baseline: 1.0012x; 1.0012x over previous
"""Trainium2 Bass kernel for nn_AttentionBase (channel attention with conv qkv).

Math restructuring (validated in fp64/fp32 numpy vs the jax reference):
  - conv1 (1x1) folds into conv2 (k=3): C_k = W2[:,:,k] @ W1  -> one k=3 conv.
  - q,k are never materialized. The per-head 16x16 channel-attention matrix
    only needs G_qk / diag(G_qq) / diag(G_kk), and all of those are sandwiches
    of the lag-autocorrelations of x:
        M_d = sum_t x_t x_{t+d}^T   (d = 0,1,2)
        G_qk = sum_{k,l} Cq_k M_{l-k} Ck_l^T  - two rank-1 boundary terms
    so pass 1 is just per-tile PE transposes of x (via identity matmul) plus
    one accumulating Gram matmul per 128-token tile: 6 PE cycles/token vs 9
    for the explicit conv+gram+sumsq formulation.
  - v is never materialized:  out = Wp @ BlockDiag(A) @ v = conv(x, M @ V_k)
    with M = Wp @ BlockDiag(A) computed on-device (tiny matmuls).

Per core (1 batch element per core, 8 cores):
  pass 1: 128 tiles: 3 shifted PE transposes (bf16, exact) -> [xT0|xT1|xT2],
          one Gram matmul accumulating [M0|M1|M2] in PSUM.
  epilogue: sandwich matmuls recover G_qk/G_qq/G_kk; norms via
          exp(-0.5*ln(ss)); rank-1 rescale; per-head softmax; folded weights.
  pass 2: k=3 conv of x (bf16) with folded weights -> output.
"""

import sys

import numpy as np

sys.path.insert(0, "/opt/trn_rl_repo")

import ml_dtypes  # noqa: E402

import concourse.bass as bass  # noqa: E402
import concourse.tile as tile  # noqa: E402
from concourse import bacc, mybir  # noqa: E402
from concourse.bass_utils import run_bass_kernel_spmd  # noqa: E402

F32 = mybir.dt.float32
BF16 = mybir.dt.bfloat16

B, C, N = 8, 128, 16384
HEADS, HD = 8, 16
NCORES = 8
CHUNK = 1024          # DMA / bf16-convert chunk (tokens)
NT = N // 128         # pass-1 token tiles
T2 = 512              # pass-2 token tile
PADW = N + 4          # col j of xbf <-> x[:, j-1]; col 0 and cols N+1.. are 0
AFT = mybir.ActivationFunctionType


def build_program():
    nc = bacc.Bacc(None, target_bir_lowering=False)

    x_d = nc.dram_tensor("x", [C, N], F32, kind="ExternalInput")
    ident_d = nc.dram_tensor("ident", [C, C], BF16, kind="ExternalInput")
    idf_d = nc.dram_tensor("idf", [C, C], F32, kind="ExternalInput")
    cqt_d = nc.dram_tensor("cqt", [C, 3 * C], BF16, kind="ExternalInput")
    ckt_d = nc.dram_tensor("ckt", [C, 3 * C], BF16, kind="ExternalInput")
    s1rhs_d = nc.dram_tensor("s1rhs", [C, 3 * 256], BF16, kind="ExternalInput")
    wv_d = nc.dram_tensor("wv", [C, 3 * C], BF16, kind="ExternalInput")
    wpt_d = nc.dram_tensor("wpt", [C, C], BF16, kind="ExternalInput")
    svec_d = nc.dram_tensor("svec", [1, C], F32, kind="ExternalInput")
    mask_d = nc.dram_tensor("mask", [C, C], F32, kind="ExternalInput")
    out_d = nc.dram_tensor("out", [C, N], F32, kind="ExternalOutput")

    with tile.TileContext(nc) as tc:
        with (
            tc.tile_pool(name="const", bufs=1) as const,
            tc.tile_pool(name="xpool", bufs=1) as xpool,
            tc.tile_pool(name="work", bufs=3) as work,
            tc.tile_pool(name="epi", bufs=1) as epi,
            tc.tile_pool(name="psum", bufs=1, space="PSUM") as psum,
        ):
            # ---- constants needed by pass 1 ----
            ident_sb = const.tile([C, C], BF16)
            nc.scalar.dma_start(out=ident_sb, in_=ident_d[:, :])
            ones_sb = const.tile([C, 1], BF16)
            nc.vector.memset(ones_sb, 1.0)
            zeros_sb = const.tile([C, C], BF16)
            nc.vector.memset(zeros_sb, 0.0)
            # Pre-warm the PE's HAM clock gate during the DMA prologue:
            # dependency-free full-width zero matmuls keep the activity
            # window busy so pass 1 starts at 2.4 GHz instead of 1.2.
            warm_ps = psum.tile([C, C], F32, tag="mps", padded_shape=[C, 3 * C])
            for _ in range(72):
                nc.tensor.matmul(warm_ps, lhsT=zeros_sb, rhs=zeros_sb)

            def warm_burst(tie_ap, n=4):
                # one matmul gated on `tie_ap` (so it fires mid-epilogue, not
                # early), then dependency-free zero matmuls that run back to
                # back -- bridges PE-idle stretches that would re-throttle HAM
                wb_ps = psum.tile([C, C], F32, tag="mps", name="wb_ps")
                kp = tie_ap.shape[0]
                nc.tensor.matmul(
                    wb_ps[0 : tie_ap.shape[1], :],
                    lhsT=tie_ap,
                    rhs=zeros_sb[0:kp, :],
                )
                for _ in range(n - 1):
                    nc.tensor.matmul(wb_ps, lhsT=zeros_sb, rhs=zeros_sb)

            # Pin ONE activation table set covering every ACT function used
            # (square/ln/exp/copy all live in natural_log_exp_and_others) so
            # no mid-kernel table reloads land on the epilogue critical path.
            from concourse.hw_specs import get_activation_tables

            tables = get_activation_tables(nc.m.arch)
            set_id = list(tables).index("natural_log_exp_and_others")
            need = {AFT.Square, AFT.Ln, AFT.Exp, AFT.Copy}
            assert need <= tables["natural_log_exp_and_others"], (
                tables["natural_log_exp_and_others"]
            )
            nc.scalar.add_instruction(
                mybir.InstLoadActFuncSet(
                    name=nc.get_next_instruction_name(),
                    ins=[],
                    outs=[],
                    act_func_set_id=set_id,
                )
            )

            # ---- x resident in SBUF as bf16, padded [C, PADW] ----
            xbf_sb = xpool.tile([C, PADW], BF16)
            nc.vector.memset(xbf_sb[:, 0:1], 0.0)
            nc.vector.memset(xbf_sb[:, N + 1 : PADW], 0.0)
            bounds = [0, 160, 320, 640, 1152]
            while bounds[-1] < N:
                bounds.append(min(N, bounds[-1] + CHUNK))
            for ci in range(len(bounds) - 1):
                a, bnd = bounds[ci], bounds[ci + 1]
                stg = work.tile([C, CHUNK], F32, tag="stg")
                nc.sync.dma_start(out=stg[:, 0 : bnd - a], in_=x_d[:, a:bnd])
                # split big casts into 512-token pieces: a cast insertion in
                # the in-order DVE/ACT queue then blocks the xt-copy cadence
                # (which gates the PE via PSUM-buffer reuse) for at most
                # ~0.55us instead of ~0.85us
                pieces = (
                    [(0, bnd - a)]
                    if bnd - a <= 512
                    else [(0, 512), (512, bnd - a)]
                )
                for lo, hi in pieces:
                    if ci % 2 == 0:
                        nc.vector.tensor_copy(
                            out=xbf_sb[:, 1 + a + lo : 1 + a + hi],
                            in_=stg[:, lo:hi],
                        )
                    else:
                        nc.scalar.copy(
                            out=xbf_sb[:, 1 + a + lo : 1 + a + hi],
                            in_=stg[:, lo:hi],
                        )

            # ---- pass 1: per-tile shifted transposes + lag-Gram accumulate --
            # tile m covers tokens [128m, 128m+128); transpose d gives rows
            # p -> x_{128m+p+d}. Gram matmul lhsT=xT0, rhs=[xT0|xT1|xT2]
            # accumulates [M0|M1|M2] with M_d = sum_t x_t x_{t+d}^T.
            LAG = 5
            m_ps = psum.tile([C, 3 * C], F32, tag="mps")
            hist = {}
            epi_loaded = False
            for m in range(NT + LAG):
                if m == 8 and not epi_loaded:
                    # epilogue-only weights: issued mid-pass-1 on the gpsimd
                    # queue so they stay off the prologue's critical path
                    epi_loaded = True
                    idf_sb = const.tile([C, C], F32)
                    nc.gpsimd.dma_start(out=idf_sb, in_=idf_d[:, :])
                    cqt_sb = const.tile([C, 3 * C], BF16)
                    nc.gpsimd.dma_start(out=cqt_sb, in_=cqt_d[:, :])
                    ckt_sb = const.tile([C, 3 * C], BF16)
                    nc.gpsimd.dma_start(out=ckt_sb, in_=ckt_d[:, :])
                    s1rhs_sb = const.tile([C, 3 * 256], BF16)
                    nc.gpsimd.dma_start(out=s1rhs_sb, in_=s1rhs_d[:, :])
                    wv_sb = const.tile([C, 3 * C], BF16)
                    nc.gpsimd.dma_start(out=wv_sb, in_=wv_d[:, :])
                    wpt_sb = const.tile([C, C], BF16)
                    nc.gpsimd.dma_start(out=wpt_sb, in_=wpt_d[:, :])
                    svec_sb = const.tile([1, C], F32)
                    nc.gpsimd.dma_start(out=svec_sb, in_=svec_d[:, :])
                    mask_sb = const.tile([C, C], F32)
                    nc.gpsimd.dma_start(out=mask_sb, in_=mask_d[:, :])
                if m < NT:
                    o = 1 + 128 * m
                    tp_ps = psum.tile([C, T2], F32, tag="tp", bufs=4)
                    for dd in range(3):
                        nc.tensor.matmul(
                            tp_ps[:, dd * C : (dd + 1) * C],
                            lhsT=xbf_sb[:, o + dd : o + dd + 128],
                            rhs=ident_sb,
                            start=True,
                            stop=True,
                        )
                    xt_sb = work.tile([C, 3 * C], BF16, tag="xt", bufs=8)
                    if m % 2 == 0:
                        nc.vector.tensor_copy(out=xt_sb, in_=tp_ps[:, 0 : 3 * C])
                    else:
                        nc.scalar.copy(out=xt_sb, in_=tp_ps[:, 0 : 3 * C])
                    hist[m] = xt_sb
                if m == NT:
                    # boundary row vectors [yq0 | yk0 | yq2 | yk2]:
                    # yq0 = (Cq_0 x_{N-1})^T etc. Needs the last x chunk, so
                    # issued after the final tile's transposes.
                    brow_ps = psum.tile([1, 4 * C], F32, tag="epiA")
                    for i, (col, blk, wsb) in enumerate(
                        [
                            (N, 0, cqt_sb),
                            (N, 0, ckt_sb),
                            (1, 2, cqt_sb),
                            (1, 2, ckt_sb),
                        ]
                    ):
                        nc.tensor.matmul(
                            brow_ps[:, i * C : (i + 1) * C],
                            lhsT=xbf_sb[:, col : col + 1],
                            rhs=wsb[:, blk * C : (blk + 1) * C],
                            start=True,
                            stop=True,
                        )
                    brow_sb = epi.tile([1, 4 * C], BF16)
                    nc.vector.tensor_copy(out=brow_sb, in_=brow_ps)
                    bneg_sb = epi.tile([1, 4 * C], BF16)
                    nc.vector.tensor_scalar_mul(bneg_sb, brow_sb, -1.0)
                if m >= LAG:
                    q = hist.pop(m - LAG)
                    nc.tensor.matmul(
                        m_ps,
                        lhsT=q[:, 0:C],
                        rhs=q,
                        start=(m - LAG == 0),
                        stop=(m - LAG == NT - 1),
                    )

            # ---- epilogue stage 0: M -> bf16, transpose M1, M2 ----
            mb_sb = epi.tile([C, 3 * C], BF16)
            nc.vector.tensor_copy(out=mb_sb[:, 0:192], in_=m_ps[:, 0:192])
            nc.scalar.copy(out=mb_sb[:, 192:384], in_=m_ps[:, 192:384])
            def s1_lhs(delta):
                if delta == 0:
                    return mb_sb[:, 0:C]
                if delta > 0:
                    return mtb_sb[:, (delta - 1) * C : delta * C]
                return mb_sb[:, -delta * C : (-delta + 1) * C]

            rp_tag = {0: "epiA", 1: "epiB", 2: "epiC"}
            rp_ps = {}
            rp_sb = {}

            def s1_block(k):
                rp_ps[k] = psum.tile(
                    [C, 256], F32, tag=rp_tag[k], name=f"rp{k}_ps"
                )
                for i, l in enumerate((0, 1, 2)):
                    nc.tensor.matmul(
                        rp_ps[k],
                        lhsT=s1_lhs(l - k),
                        rhs=s1rhs_sb[:, l * 256 : (l + 1) * 256],
                        start=(i == 0),
                        stop=(i == 2),
                    )
                rp_sb[k] = epi.tile([C, 256], BF16, name=f"rp{k}_sb")
                if k == 2:
                    nc.vector.tensor_copy(out=rp_sb[k], in_=rp_ps[k])
                else:
                    nc.scalar.copy(out=rp_sb[k], in_=rp_ps[k])

            # k=2 uses only mb (deltas -2,-1,0): issue it BEFORE the M1T/M2T
            # transposes so the mtb cast overlaps real sandwich matmuls; k=1
            # and k=0 each put their single mtb-dependent term last.
            s1_block(2)
            mt_ps = psum.tile([C, 2 * C], F32, tag="epiB")
            nc.tensor.matmul(
                mt_ps[:, 0:C], lhsT=mb_sb[:, C : 2 * C], rhs=ident_sb,
                start=True, stop=True,
            )
            nc.tensor.matmul(
                mt_ps[:, C : 2 * C], lhsT=mb_sb[:, 2 * C : 3 * C], rhs=ident_sb,
                start=True, stop=True,
            )
            mtb_sb = epi.tile([C, 2 * C], BF16)
            nc.scalar.copy(out=mtb_sb, in_=mt_ps)
            warm_burst(mb_sb[:, 0:C], 4)
            s1_block(1)
            s1_block(0)

            # stage 2: [G_qk | G_qq] and G_kk, minus rank-1 boundary terms
            gq_ps = psum.tile([C, 256], F32, tag="epiA")
            for k in range(3):
                nc.tensor.matmul(
                    gq_ps,
                    lhsT=cqt_sb[:, k * C : (k + 1) * C],
                    rhs=rp_sb[k],
                    start=(k == 0),
                    stop=False,
                )
            # G_qk -= yq0 yk0^T + yq2 yk2^T ; G_qq -= yq0 yq0^T + yq2 yq2^T
            nc.tensor.matmul(
                gq_ps[:, 0:C], lhsT=bneg_sb[:, 0:C], rhs=brow_sb[:, C : 2 * C],
                start=False, stop=False,
            )
            nc.tensor.matmul(
                gq_ps[:, 0:C], lhsT=bneg_sb[:, 2 * C : 3 * C],
                rhs=brow_sb[:, 3 * C : 4 * C], start=False, stop=True,
            )
            nc.tensor.matmul(
                gq_ps[:, C : 2 * C], lhsT=bneg_sb[:, 0:C], rhs=brow_sb[:, 0:C],
                start=False, stop=False,
            )
            nc.tensor.matmul(
                gq_ps[:, C : 2 * C], lhsT=bneg_sb[:, 2 * C : 3 * C],
                rhs=brow_sb[:, 2 * C : 3 * C], start=False, stop=True,
            )
            gk_ps = psum.tile([C, C], F32, tag="epiC")
            for k in range(3):
                nc.tensor.matmul(
                    gk_ps,
                    lhsT=ckt_sb[:, k * C : (k + 1) * C],
                    rhs=rp_sb[k][:, 0:C],
                    start=(k == 0),
                    stop=False,
                )
            nc.tensor.matmul(
                gk_ps, lhsT=bneg_sb[:, C : 2 * C], rhs=brow_sb[:, C : 2 * C],
                start=False, stop=False,
            )
            nc.tensor.matmul(
                gk_ps, lhsT=bneg_sb[:, 3 * C : 4 * C],
                rhs=brow_sb[:, 3 * C : 4 * C], start=False, stop=True,
            )

            # norms: ss_q = diag(G_qq), ss_k = diag(G_kk) via identity mask +
            # ones-colsum matmul -> [1, 2C] row
            dqk_sb = epi.tile([C, 2 * C], BF16)
            nc.vector.tensor_mul(dqk_sb[:, 0:C], gq_ps[:, C : 2 * C], idf_sb)
            nc.vector.tensor_mul(dqk_sb[:, C : 2 * C], gk_ps, idf_sb)
            ss_ps = psum.tile([1, 2 * C], F32, tag="epiB", name="ss_ps")
            nc.tensor.matmul(ss_ps, lhsT=ones_sb, rhs=dqk_sb, start=True, stop=True)
            warm_burst(dqk_sb[:, 0:C], 6)
            # r = rsqrt(ss) via exp(-0.5*ln(ss)); ss is a large positive
            # sum of squares so no clamp is needed, and ACT reads it straight
            # from PSUM (saves a copy + a clamp on the serial chain)
            ss_sb = epi.tile([1, 2 * C], F32)
            nc.scalar.activation(ss_sb, ss_ps, AFT.Ln)
            # tiny bf16 matmuls tied to epilogue intermediates keep the PE's
            # HAM activity window warm through the serial epilogue ops
            ssb_sb = epi.tile([1, 2 * C], BF16)
            nc.vector.tensor_copy(out=ssb_sb, in_=ss_sb)
            warm_burst(ssb_sb[:, 0:C], 6)
            r_sb = epi.tile([1, 2 * C], F32)
            nc.scalar.activation(r_sb, ss_sb, AFT.Exp, scale=-0.5)
            rq_sb = epi.tile([1, C], F32)
            nc.vector.tensor_mul(rq_sb, r_sb[:, 0:C], svec_sb)
            rqb_sb = epi.tile([1, C], BF16)
            nc.vector.tensor_copy(out=rqb_sb, in_=rq_sb)
            warm_burst(rqb_sb, 6)

            outer_ps = psum.tile([C, C], F32, tag="epiB")
            nc.tensor.matmul(outer_ps, lhsT=rq_sb, rhs=r_sb[:, C : 2 * C])
            outer_sb = epi.tile([C, C], F32)
            nc.vector.tensor_copy(out=outer_sb, in_=outer_ps)

            # A = softmax over each 16x16 diagonal block; the additive mask is
            # -1e30 off-block, so exp underflows to exactly 0 there. The row
            # max is taken over the unmasked row (softmax is shift-invariant
            # and |A|<=1, so any in-range shift is numerically fine).
            a_sb = epi.tile([C, C], F32)
            nc.vector.tensor_mul(a_sb, gq_ps[:, 0:C], outer_sb)
            nc.vector.tensor_add(a_sb, a_sb, mask_sb)
            rsum = epi.tile([C, 1], F32)
            ae_sb = epi.tile([C, C], BF16)
            nc.scalar.activation(ae_sb, a_sb, AFT.Exp)
            warm_burst(ae_sb[:, 0:C], 6)
            nc.vector.reduce_sum(out=rsum, in_=ae_sb, axis=mybir.AxisListType.X)
            nc.vector.reciprocal(rsum, rsum)
            wptn_sb = epi.tile([C, C], BF16)
            nc.vector.tensor_scalar_mul(wptn_sb, wpt_sb, rsum)

            # MT[d, m] = sum_c A[c, d] * WpT[c, m]
            mtm_ps = psum.tile([C, C], F32, tag="epiC")
            nc.tensor.matmul(mtm_ps, lhsT=ae_sb, rhs=wptn_sb)
            mtm_sb = epi.tile([C, C], BF16)
            nc.vector.tensor_copy(out=mtm_sb, in_=mtm_ps)
            warm_burst(mtm_sb, 4)

            foldT_sb = epi.tile([C, 3 * C], BF16)
            fold_tag = {0: "epiC", 1: "epiB", 2: "epiC"}
            for k in range(3):
                fold_ps = psum.tile([C, C], F32, tag=fold_tag[k], name=f"fold{k}")
                nc.tensor.matmul(
                    fold_ps, lhsT=wv_sb[:, k * C : (k + 1) * C], rhs=mtm_sb
                )
                nc.vector.tensor_copy(
                    out=foldT_sb[:, k * C : (k + 1) * C], in_=fold_ps
                )

            # ---- pass 2: folded k=3 conv of x (bf16), channel-major ----
            for j in range(N // T2):
                o_ps = psum.tile([C, T2], F32, tag="tp", bufs=4)
                for k in range(3):
                    o = j * T2 + k
                    nc.tensor.matmul(
                        o_ps,
                        lhsT=foldT_sb[:, k * C : (k + 1) * C],
                        rhs=xbf_sb[:, o : o + T2],
                        start=(k == 0),
                        stop=(k == 2),
                    )
                o_sb = work.tile([C, T2], F32, tag="o_sb", bufs=8)
                if j % 2 == 0:
                    nc.vector.tensor_copy(out=o_sb, in_=o_ps)
                else:
                    nc.scalar.copy(out=o_sb, in_=o_ps)
                nc.sync.dma_start(
                    out=out_d[:, j * T2 : (j + 1) * T2], in_=o_sb
                )

    nc.finalize()
    return nc


def _prep_weights(w_qkv1, w_qkv2, w_proj, scale):
    W1 = np.asarray(w_qkv1, np.float32)[:, :, 0]          # [384, 128]
    W2 = np.asarray(w_qkv2, np.float32)                   # [384, 384, 3]
    Ck = np.stack([W2[:, :, k] @ W1 for k in range(3)])   # [3, 384, 128]
    Qk, Kk, Vk = Ck[:, 0:C, :], Ck[:, C : 2 * C, :], Ck[:, 2 * C :, :]
    bf = ml_dtypes.bfloat16
    cqt = np.concatenate([Qk[k].T for k in range(3)], axis=1)   # [128, 384]
    ckt = np.concatenate([Kk[k].T for k in range(3)], axis=1)
    s1rhs = np.concatenate(
        [np.concatenate([Kk[l].T, Qk[l].T], axis=1) for l in range(3)], axis=1
    )                                                      # [128, 768]
    wv = np.concatenate([Vk[k] for k in range(3)], axis=1)  # [128, 384]
    wpt = np.ascontiguousarray(np.asarray(w_proj, np.float32)[:, :, 0].T)
    svec = np.repeat(np.asarray(scale, np.float32)[:, 0, 0], HD)[None, :]
    mask = np.full((C, C), -1e30, np.float32)
    for h in range(HEADS):
        mask[h * HD : (h + 1) * HD, h * HD : (h + 1) * HD] = 0.0
    return {
        "ident": np.eye(C, dtype=bf),
        "idf": np.eye(C, dtype=np.float32),
        "cqt": np.ascontiguousarray(cqt).astype(bf),
        "ckt": np.ascontiguousarray(ckt).astype(bf),
        "s1rhs": np.ascontiguousarray(s1rhs).astype(bf),
        "wv": np.ascontiguousarray(wv).astype(bf),
        "wpt": wpt.astype(bf),
        "svec": np.ascontiguousarray(svec, np.float32),
        "mask": mask,
    }


_CACHE = {}


def kernel(x, w_qkv1, w_qkv2, w_proj, scale, _trace=False, _tmpdir=None):
    x = np.asarray(x, np.float32)
    assert x.shape == (B, C, N), x.shape
    wmap = _prep_weights(w_qkv1, w_qkv2, w_proj, scale)

    if "nc" not in _CACHE:
        _CACHE["nc"] = build_program()
    nc = _CACHE["nc"]

    in_maps = [
        {"x": np.ascontiguousarray(x[i]), **wmap}
        for i in range(NCORES)
    ]
    res = run_bass_kernel_spmd(
        nc,
        in_maps,
        core_ids=list(range(NCORES)),
        trace=_trace,
        tmpdir=_tmpdir,
    )
    out = np.stack([r["out"] for r in res.results]).astype(np.float32)
    if _trace:
        _CACHE["last_result"] = res
    return out


# revision 28
# speedup vs baseline: 1.0199x; 1.0187x over previous
"""Trainium2 Bass kernel for nn_AttentionBase (channel attention with conv qkv).

Math restructuring (validated in fp64/fp32 numpy vs the jax reference):
  - conv1 (1x1) folds into conv2 (k=3): C_k = W2[:,:,k] @ W1  -> one k=3 conv.
  - q,k are never materialized. The per-head 16x16 channel-attention matrix
    only needs G_qk / diag(G_qq) / diag(G_kk), and all of those are sandwiches
    of the lag-autocorrelations of x:
        M_d = sum_t x_t x_{t+d}^T   (d = 0,1,2)
        G_qk = sum_{k,l} Cq_k M_{l-k} Ck_l^T  - two rank-1 boundary terms
    so pass 1 is just per-tile PE transposes of x (via identity matmul) plus
    one accumulating Gram matmul per 128-token tile: 6 PE cycles/token vs 9
    for the explicit conv+gram+sumsq formulation.
  - v is never materialized:  out = Wp @ BlockDiag(A) @ v = conv(x, M @ V_k)
    with M = Wp @ BlockDiag(A) computed on-device (tiny matmuls).

Per core (1 batch element per core, 8 cores):
  pass 1: 128 tiles: 3 shifted PE transposes (bf16, exact) -> [xT0|xT1|xT2],
          one Gram matmul accumulating [M0|M1|M2] in PSUM.
  epilogue: sandwich matmuls recover G_qk/G_qq/G_kk; norms via
          exp(-0.5*ln(ss)); rank-1 rescale; per-head softmax; folded weights.
  pass 2: k=3 conv of x (bf16) with folded weights -> output.
"""

import sys

import numpy as np

sys.path.insert(0, "/opt/trn_rl_repo")

import ml_dtypes  # noqa: E402

import concourse.bass as bass  # noqa: E402
import concourse.tile as tile  # noqa: E402
from concourse import bacc, mybir  # noqa: E402
from concourse.bass_utils import run_bass_kernel_spmd  # noqa: E402

F32 = mybir.dt.float32
BF16 = mybir.dt.bfloat16

B, C, N = 8, 128, 16384
HEADS, HD = 8, 16
NCORES = 8
CHUNK = 1024          # DMA / bf16-convert chunk (tokens)
NT = N // 128         # pass-1 token tiles
T2 = 512              # pass-2 token tile
PADW = N + 4          # col j of xbf <-> x[:, j-1]; col 0 and cols N+1.. are 0
AFT = mybir.ActivationFunctionType


def build_program():
    nc = bacc.Bacc(None, target_bir_lowering=False)

    x_d = nc.dram_tensor("x", [C, N], F32, kind="ExternalInput")
    ident_d = nc.dram_tensor("ident", [C, C], BF16, kind="ExternalInput")
    idf_d = nc.dram_tensor("idf", [C, C], F32, kind="ExternalInput")
    cqt_d = nc.dram_tensor("cqt", [C, 3 * C], BF16, kind="ExternalInput")
    ckt_d = nc.dram_tensor("ckt", [C, 3 * C], BF16, kind="ExternalInput")
    s1rhs_d = nc.dram_tensor("s1rhs", [C, 3 * 256], BF16, kind="ExternalInput")
    wv_d = nc.dram_tensor("wv", [C, 3 * C], BF16, kind="ExternalInput")
    wpt_d = nc.dram_tensor("wpt", [C, C], BF16, kind="ExternalInput")
    svec_d = nc.dram_tensor("svec", [1, C], F32, kind="ExternalInput")
    mask_d = nc.dram_tensor("mask", [C, C], F32, kind="ExternalInput")
    out_d = nc.dram_tensor("out", [C, N], F32, kind="ExternalOutput")

    with tile.TileContext(nc) as tc:
        with (
            tc.tile_pool(name="const", bufs=1) as const,
            tc.tile_pool(name="xpool", bufs=1) as xpool,
            tc.tile_pool(name="work", bufs=3) as work,
            tc.tile_pool(name="epi", bufs=1) as epi,
            tc.tile_pool(name="psum", bufs=1, space="PSUM") as psum,
        ):
            # ---- constants needed by pass 1 ----
            ident_sb = const.tile([C, C], BF16)
            nc.scalar.dma_start(out=ident_sb, in_=ident_d[:, :])
            ones_sb = const.tile([C, 1], BF16)
            nc.vector.memset(ones_sb, 1.0)
            zeros_sb = const.tile([C, C], BF16)
            nc.vector.memset(zeros_sb, 0.0)
            # Pre-warm the PE's HAM clock gate during the DMA prologue:
            # dependency-free full-width zero matmuls keep the activity
            # window busy so pass 1 starts at 2.4 GHz instead of 1.2.
            warm_ps = psum.tile([C, C], F32, tag="mps", padded_shape=[C, 3 * C])
            for _ in range(72):
                nc.tensor.matmul(warm_ps, lhsT=zeros_sb, rhs=zeros_sb)

            def warm_burst(tie_ap, n=4):
                # one matmul gated on `tie_ap` (so it fires mid-epilogue, not
                # early), then dependency-free zero matmuls that run back to
                # back -- bridges PE-idle stretches that would re-throttle HAM
                wb_ps = psum.tile([C, C], F32, tag="mps", name="wb_ps")
                kp = tie_ap.shape[0]
                nc.tensor.matmul(
                    wb_ps[0 : tie_ap.shape[1], :],
                    lhsT=tie_ap,
                    rhs=zeros_sb[0:kp, :],
                )
                for _ in range(n - 1):
                    nc.tensor.matmul(wb_ps, lhsT=zeros_sb, rhs=zeros_sb)

            # Pin ONE activation table set covering every ACT function used
            # (square/ln/exp/copy all live in natural_log_exp_and_others) so
            # no mid-kernel table reloads land on the epilogue critical path.
            from concourse.hw_specs import get_activation_tables

            tables = get_activation_tables(nc.m.arch)
            set_id = list(tables).index("natural_log_exp_and_others")
            need = {AFT.Square, AFT.Ln, AFT.Exp, AFT.Copy}
            assert need <= tables["natural_log_exp_and_others"], (
                tables["natural_log_exp_and_others"]
            )
            nc.scalar.add_instruction(
                mybir.InstLoadActFuncSet(
                    name=nc.get_next_instruction_name(),
                    ins=[],
                    outs=[],
                    act_func_set_id=set_id,
                )
            )

            # ---- x resident in SBUF as bf16, padded [C, PADW] ----
            xbf_sb = xpool.tile([C, PADW], BF16)
            nc.vector.memset(xbf_sb[:, 0:1], 0.0)
            nc.vector.memset(xbf_sb[:, N + 1 : PADW], 0.0)
            bounds = [0, 160, 320, 640, 1152]
            while bounds[-1] < N:
                bounds.append(min(N, bounds[-1] + CHUNK))
            for ci in range(len(bounds) - 1):
                a, bnd = bounds[ci], bounds[ci + 1]
                stg = work.tile([C, CHUNK], F32, tag="stg", bufs=4)
                nc.sync.dma_start(out=stg[:, 0 : bnd - a], in_=x_d[:, a:bnd])
                # split big casts into 512-token pieces: a cast insertion in
                # the in-order DVE/ACT queue then blocks the xt-copy cadence
                # (which gates the PE via PSUM-buffer reuse) for at most
                # ~0.55us instead of ~0.85us
                pieces = (
                    [(0, bnd - a)]
                    if bnd - a <= 512
                    else [(0, 512), (512, bnd - a)]
                )
                for lo, hi in pieces:
                    if ci % 2 == 0:
                        nc.vector.tensor_copy(
                            out=xbf_sb[:, 1 + a + lo : 1 + a + hi],
                            in_=stg[:, lo:hi],
                        )
                    else:
                        nc.scalar.copy(
                            out=xbf_sb[:, 1 + a + lo : 1 + a + hi],
                            in_=stg[:, lo:hi],
                        )

            # ---- pass 1: per-tile shifted transposes + lag-Gram accumulate --
            # tile m covers tokens [128m, 128m+128); transpose d gives rows
            # p -> x_{128m+p+d}. Gram matmul lhsT=xT0, rhs=[xT0|xT1|xT2]
            # accumulates [M0|M1|M2] with M_d = sum_t x_t x_{t+d}^T.
            LAG = 5
            m_ps = psum.tile([C, 3 * C], F32, tag="mps")
            hist = {}
            epi_loaded = False
            for m in range(NT + LAG):
                if m == 8 and not epi_loaded:
                    # epilogue-only weights: issued mid-pass-1 on the gpsimd
                    # queue so they stay off the prologue's critical path
                    epi_loaded = True
                    idf_sb = const.tile([C, C], F32)
                    nc.gpsimd.dma_start(out=idf_sb, in_=idf_d[:, :])
                    cqt_sb = const.tile([C, 3 * C], BF16)
                    nc.gpsimd.dma_start(out=cqt_sb, in_=cqt_d[:, :])
                    ckt_sb = const.tile([C, 3 * C], BF16)
                    nc.gpsimd.dma_start(out=ckt_sb, in_=ckt_d[:, :])
                    s1rhs_sb = const.tile([C, 3 * 256], BF16)
                    nc.gpsimd.dma_start(out=s1rhs_sb, in_=s1rhs_d[:, :])
                    wv_sb = const.tile([C, 3 * C], BF16)
                    nc.gpsimd.dma_start(out=wv_sb, in_=wv_d[:, :])
                    wpt_sb = const.tile([C, C], BF16)
                    nc.gpsimd.dma_start(out=wpt_sb, in_=wpt_d[:, :])
                    svec_sb = const.tile([1, C], F32)
                    nc.gpsimd.dma_start(out=svec_sb, in_=svec_d[:, :])
                    mask_sb = const.tile([C, C], F32)
                    nc.gpsimd.dma_start(out=mask_sb, in_=mask_d[:, :])
                if m < NT:
                    o = 1 + 128 * m
                    tp_ps = psum.tile([C, T2], F32, tag="tp", bufs=4)
                    for dd in range(3):
                        nc.tensor.matmul(
                            tp_ps[:, dd * C : (dd + 1) * C],
                            lhsT=xbf_sb[:, o + dd : o + dd + 128],
                            rhs=ident_sb,
                            start=True,
                            stop=True,
                        )
                    xt_sb = work.tile([C, 3 * C], BF16, tag="xt", bufs=12)
                    if m % 2 == 0:
                        nc.vector.tensor_copy(out=xt_sb, in_=tp_ps[:, 0 : 3 * C])
                    else:
                        nc.scalar.copy(out=xt_sb, in_=tp_ps[:, 0 : 3 * C])
                    hist[m] = xt_sb
                if m == NT:
                    # boundary row vectors [yq0 | yk0 | yq2 | yk2]:
                    # yq0 = (Cq_0 x_{N-1})^T etc. Needs the last x chunk, so
                    # issued after the final tile's transposes.
                    brow_ps = psum.tile([1, 4 * C], F32, tag="epiA")
                    for i, (col, blk, wsb) in enumerate(
                        [
                            (N, 0, cqt_sb),
                            (N, 0, ckt_sb),
                            (1, 2, cqt_sb),
                            (1, 2, ckt_sb),
                        ]
                    ):
                        nc.tensor.matmul(
                            brow_ps[:, i * C : (i + 1) * C],
                            lhsT=xbf_sb[:, col : col + 1],
                            rhs=wsb[:, blk * C : (blk + 1) * C],
                            start=True,
                            stop=True,
                        )
                    brow_sb = epi.tile([1, 4 * C], BF16)
                    nc.vector.tensor_copy(out=brow_sb, in_=brow_ps)
                    bneg_sb = epi.tile([1, 4 * C], BF16)
                    nc.vector.tensor_scalar_mul(bneg_sb, brow_sb, -1.0)
                if m >= LAG:
                    q = hist.pop(m - LAG)
                    nc.tensor.matmul(
                        m_ps,
                        lhsT=q[:, 0:C],
                        rhs=q,
                        start=(m - LAG == 0),
                        stop=(m - LAG == NT - 1),
                    )

            # ---- epilogue stage 0: M -> bf16, transpose M1, M2 ----
            mb_sb = epi.tile([C, 3 * C], BF16)
            nc.vector.tensor_copy(out=mb_sb[:, 0:192], in_=m_ps[:, 0:192])
            nc.scalar.copy(out=mb_sb[:, 192:384], in_=m_ps[:, 192:384])
            def s1_lhs(delta):
                if delta == 0:
                    return mb_sb[:, 0:C]
                if delta > 0:
                    return mtb_sb[:, (delta - 1) * C : delta * C]
                return mb_sb[:, -delta * C : (-delta + 1) * C]

            rp_tag = {0: "epiA", 1: "epiB", 2: "epiC"}
            rp_ps = {}
            rp_sb = {}

            def s1_block(k):
                rp_ps[k] = psum.tile(
                    [C, 256], F32, tag=rp_tag[k], name=f"rp{k}_ps"
                )
                for i, l in enumerate((0, 1, 2)):
                    nc.tensor.matmul(
                        rp_ps[k],
                        lhsT=s1_lhs(l - k),
                        rhs=s1rhs_sb[:, l * 256 : (l + 1) * 256],
                        start=(i == 0),
                        stop=(i == 2),
                    )
                rp_sb[k] = epi.tile([C, 256], BF16, name=f"rp{k}_sb")
                if k == 2:
                    nc.vector.tensor_copy(out=rp_sb[k], in_=rp_ps[k])
                else:
                    nc.scalar.copy(out=rp_sb[k], in_=rp_ps[k])

            # k=2 uses only mb (deltas -2,-1,0): issue it BEFORE the M1T/M2T
            # transposes so the mtb cast overlaps real sandwich matmuls; k=1
            # and k=0 each put their single mtb-dependent term last.
            s1_block(2)
            mt_ps = psum.tile([C, 2 * C], F32, tag="epiB")
            nc.tensor.matmul(
                mt_ps[:, 0:C], lhsT=mb_sb[:, C : 2 * C], rhs=ident_sb,
                start=True, stop=True,
            )
            nc.tensor.matmul(
                mt_ps[:, C : 2 * C], lhsT=mb_sb[:, 2 * C : 3 * C], rhs=ident_sb,
                start=True, stop=True,
            )
            mtb_sb = epi.tile([C, 2 * C], BF16)
            nc.scalar.copy(out=mtb_sb, in_=mt_ps)
            warm_burst(mb_sb[:, 0:C], 4)
            s1_block(1)
            s1_block(0)

            # stage 2: [G_qk | G_qq] and G_kk, minus rank-1 boundary terms
            gq_ps = psum.tile([C, 256], F32, tag="epiA")
            for k in range(3):
                nc.tensor.matmul(
                    gq_ps,
                    lhsT=cqt_sb[:, k * C : (k + 1) * C],
                    rhs=rp_sb[k],
                    start=(k == 0),
                    stop=False,
                )
            # G_qk -= yq0 yk0^T + yq2 yk2^T ; G_qq -= yq0 yq0^T + yq2 yq2^T
            nc.tensor.matmul(
                gq_ps[:, 0:C], lhsT=bneg_sb[:, 0:C], rhs=brow_sb[:, C : 2 * C],
                start=False, stop=False,
            )
            nc.tensor.matmul(
                gq_ps[:, 0:C], lhsT=bneg_sb[:, 2 * C : 3 * C],
                rhs=brow_sb[:, 3 * C : 4 * C], start=False, stop=True,
            )
            nc.tensor.matmul(
                gq_ps[:, C : 2 * C], lhsT=bneg_sb[:, 0:C], rhs=brow_sb[:, 0:C],
                start=False, stop=False,
            )
            nc.tensor.matmul(
                gq_ps[:, C : 2 * C], lhsT=bneg_sb[:, 2 * C : 3 * C],
                rhs=brow_sb[:, 2 * C : 3 * C], start=False, stop=True,
            )
            gk_ps = psum.tile([C, C], F32, tag="epiC")
            for k in range(3):
                nc.tensor.matmul(
                    gk_ps,
                    lhsT=ckt_sb[:, k * C : (k + 1) * C],
                    rhs=rp_sb[k][:, 0:C],
                    start=(k == 0),
                    stop=False,
                )
            nc.tensor.matmul(
                gk_ps, lhsT=bneg_sb[:, C : 2 * C], rhs=brow_sb[:, C : 2 * C],
                start=False, stop=False,
            )
            nc.tensor.matmul(
                gk_ps, lhsT=bneg_sb[:, 3 * C : 4 * C],
                rhs=brow_sb[:, 3 * C : 4 * C], start=False, stop=True,
            )

            # norms: ss_q = diag(G_qq), ss_k = diag(G_kk) via identity mask +
            # ones-colsum matmul -> [1, 2C] row
            dqk_sb = epi.tile([C, 2 * C], BF16)
            nc.vector.tensor_mul(dqk_sb[:, 0:C], gq_ps[:, C : 2 * C], idf_sb)
            nc.vector.tensor_mul(dqk_sb[:, C : 2 * C], gk_ps, idf_sb)
            ss_ps = psum.tile([1, 2 * C], F32, tag="epiB", name="ss_ps")
            nc.tensor.matmul(ss_ps, lhsT=ones_sb, rhs=dqk_sb, start=True, stop=True)
            warm_burst(dqk_sb[:, 0:C], 6)
            # r = rsqrt(ss) via exp(-0.5*ln(ss)); ss is a large positive
            # sum of squares so no clamp is needed, and ACT reads it straight
            # from PSUM (saves a copy + a clamp on the serial chain)
            ss_sb = epi.tile([1, 2 * C], F32)
            nc.scalar.activation(ss_sb, ss_ps, AFT.Ln)
            # tiny bf16 matmuls tied to epilogue intermediates keep the PE's
            # HAM activity window warm through the serial epilogue ops
            ssb_sb = epi.tile([1, 2 * C], BF16)
            nc.vector.tensor_copy(out=ssb_sb, in_=ss_sb)
            warm_burst(ssb_sb[:, 0:C], 6)
            r_sb = epi.tile([1, 2 * C], F32)
            nc.scalar.activation(r_sb, ss_sb, AFT.Exp, scale=-0.5)
            rq_sb = epi.tile([1, C], F32)
            nc.vector.tensor_mul(rq_sb, r_sb[:, 0:C], svec_sb)
            rqb_sb = epi.tile([1, C], BF16)
            nc.vector.tensor_copy(out=rqb_sb, in_=rq_sb)
            warm_burst(rqb_sb, 6)

            outer_ps = psum.tile([C, C], F32, tag="epiB")
            nc.tensor.matmul(outer_ps, lhsT=rq_sb, rhs=r_sb[:, C : 2 * C])
            outer_sb = epi.tile([C, C], F32)
            nc.vector.tensor_copy(out=outer_sb, in_=outer_ps)

            # A = softmax over each 16x16 diagonal block; the additive mask is
            # -1e30 off-block, so exp underflows to exactly 0 there. The row
            # max is taken over the unmasked row (softmax is shift-invariant
            # and |A|<=1, so any in-range shift is numerically fine).
            a_sb = epi.tile([C, C], F32)
            nc.vector.tensor_mul(a_sb, gq_ps[:, 0:C], outer_sb)
            nc.vector.tensor_add(a_sb, a_sb, mask_sb)
            rsum = epi.tile([C, 1], F32)
            ae_sb = epi.tile([C, C], BF16)
            nc.scalar.activation(ae_sb, a_sb, AFT.Exp)
            warm_burst(ae_sb[:, 0:C], 6)
            nc.vector.reduce_sum(out=rsum, in_=ae_sb, axis=mybir.AxisListType.X)
            nc.vector.reciprocal(rsum, rsum)
            wptn_sb = epi.tile([C, C], BF16)
            nc.vector.tensor_scalar_mul(wptn_sb, wpt_sb, rsum)

            # MT[d, m] = sum_c A[c, d] * WpT[c, m]
            mtm_ps = psum.tile([C, C], F32, tag="epiC")
            nc.tensor.matmul(mtm_ps, lhsT=ae_sb, rhs=wptn_sb)
            mtm_sb = epi.tile([C, C], BF16)
            nc.vector.tensor_copy(out=mtm_sb, in_=mtm_ps)
            warm_burst(mtm_sb, 4)

            foldT_sb = epi.tile([C, 3 * C], BF16)
            fold_tag = {0: "epiC", 1: "epiB", 2: "epiC"}
            for k in range(3):
                fold_ps = psum.tile([C, C], F32, tag=fold_tag[k], name=f"fold{k}")
                nc.tensor.matmul(
                    fold_ps, lhsT=wv_sb[:, k * C : (k + 1) * C], rhs=mtm_sb
                )
                nc.vector.tensor_copy(
                    out=foldT_sb[:, k * C : (k + 1) * C], in_=fold_ps
                )

            # ---- pass 2: folded k=3 conv of x (bf16), channel-major ----
            for j in range(N // T2):
                o_ps = psum.tile([C, T2], F32, tag="tp", bufs=4)
                for k in range(3):
                    o = j * T2 + k
                    nc.tensor.matmul(
                        o_ps,
                        lhsT=foldT_sb[:, k * C : (k + 1) * C],
                        rhs=xbf_sb[:, o : o + T2],
                        start=(k == 0),
                        stop=(k == 2),
                    )
                o_sb = work.tile([C, T2], F32, tag="o_sb", bufs=10)
                if j % 2 == 0:
                    nc.vector.tensor_copy(out=o_sb, in_=o_ps)
                else:
                    nc.scalar.copy(out=o_sb, in_=o_ps)
                nc.sync.dma_start(
                    out=out_d[:, j * T2 : (j + 1) * T2], in_=o_sb
                )

    nc.finalize()
    return nc


def _prep_weights(w_qkv1, w_qkv2, w_proj, scale):
    W1 = np.asarray(w_qkv1, np.float32)[:, :, 0]          # [384, 128]
    W2 = np.asarray(w_qkv2, np.float32)                   # [384, 384, 3]
    Ck = np.stack([W2[:, :, k] @ W1 for k in range(3)])   # [3, 384, 128]
    Qk, Kk, Vk = Ck[:, 0:C, :], Ck[:, C : 2 * C, :], Ck[:, 2 * C :, :]
    bf = ml_dtypes.bfloat16
    cqt = np.concatenate([Qk[k].T for k in range(3)], axis=1)   # [128, 384]
    ckt = np.concatenate([Kk[k].T for k in range(3)], axis=1)
    s1rhs = np.concatenate(
        [np.concatenate([Kk[l].T, Qk[l].T], axis=1) for l in range(3)], axis=1
    )                                                      # [128, 768]
    wv = np.concatenate([Vk[k] for k in range(3)], axis=1)  # [128, 384]
    wpt = np.ascontiguousarray(np.asarray(w_proj, np.float32)[:, :, 0].T)
    svec = np.repeat(np.asarray(scale, np.float32)[:, 0, 0], HD)[None, :]
    mask = np.full((C, C), -1e30, np.float32)
    for h in range(HEADS):
        mask[h * HD : (h + 1) * HD, h * HD : (h + 1) * HD] = 0.0
    return {
        "ident": np.eye(C, dtype=bf),
        "idf": np.eye(C, dtype=np.float32),
        "cqt": np.ascontiguousarray(cqt).astype(bf),
        "ckt": np.ascontiguousarray(ckt).astype(bf),
        "s1rhs": np.ascontiguousarray(s1rhs).astype(bf),
        "wv": np.ascontiguousarray(wv).astype(bf),
        "wpt": wpt.astype(bf),
        "svec": np.ascontiguousarray(svec, np.float32),
        "mask": mask,
    }


_CACHE = {}


def kernel(x, w_qkv1, w_qkv2, w_proj, scale, _trace=False, _tmpdir=None):
    x = np.asarray(x, np.float32)
    assert x.shape == (B, C, N), x.shape
    wmap = _prep_weights(w_qkv1, w_qkv2, w_proj, scale)

    if "nc" not in _CACHE:
        _CACHE["nc"] = build_program()
    nc = _CACHE["nc"]

    in_maps = [
        {"x": np.ascontiguousarray(x[i]), **wmap}
        for i in range(NCORES)
    ]
    res = run_bass_kernel_spmd(
        nc,
        in_maps,
        core_ids=list(range(NCORES)),
        trace=_trace,
        tmpdir=_tmpdir,
    )
    out = np.stack([r["out"] for r in res.results]).astype(np.float32)
    if _trace:
        _CACHE["last_result"] = res
    return out
